# revision 22
# baseline (speedup 1.0000x reference)
"""CrossAttention kernel for 8 TRN2 NeuronCores.

Data-parallel over batch B=8: core b computes batch b entirely on-chip.
Per core (x_b [4096, 1024]):
  xT (PE transpose) -> K^T = Wk.T @ xT (fp32r), V = xT.T @ Wv (fp32r, bf16 out)
  sim = bd_q.T @ K^T per head-pair (block-diagonal packed q, fp32r)
  exp (no-max softmax: scores ~ N(0,1), fp32 exp cannot overflow; row sums
  via activation accum_out), attn^T via PE transpose (bf16), AV (bf16),
  out-projection + layernorm + residual epilogue.
"""

import os
import sys

for _p in (
    "/root/.axon_site",
    "/root/.axon_site/_ro/trn_rl_repo",
    "/root/.axon_site/_ro/pypackages",
    "/opt/trn_rl_repo",
):
    if os.path.isdir(_p) and _p not in sys.path:
        sys.path.append(_p)

from contextlib import ExitStack

import numpy as np

import concourse.bass as bass
from concourse import bacc
import concourse.mybir as mybir
import concourse.tile as tile
from concourse import masks
from concourse.bass_utils import run_bass_kernel_spmd

F32 = mybir.dt.float32
F32R = mybir.dt.float32r
BF16 = mybir.dt.bfloat16
AX = mybir.AxisListType
AF = mybir.ActivationFunctionType
OP = mybir.AluOpType

B, S, D = 8, 4096, 1024          # batch, seq, d_in (= d_out = qk_dim = v_dim)
L, DLAT = 64, 512                # latents
H, DH = 16, 64                   # heads
NP = 8                           # head pairs (2 heads = 128 partitions)
DB = 8                           # d_in blocks of 128
NCH, SC = 8, 512                 # s-chunks
SCALE = DH ** -0.5
RSQRT2 = 2 ** -0.5
LN_EPS = 1e-5
N_CORES = 8

LAST_RESULT = None


def build_nc(debug_taps=False):
    nc = bacc.Bacc(
        "TRN2", target_bir_lowering=False, debug=False, num_devices=N_CORES
    )
    x_d = nc.declare_dram_parameter("x", [S, D], F32, isOutput=False)
    lat_d = nc.declare_dram_parameter("latents", [L, DLAT], F32, isOutput=False)
    wq_d = nc.declare_dram_parameter("Wq", [DLAT, D], F32R, isOutput=False)
    bq_d = nc.declare_dram_parameter("bq", [D], F32, isOutput=False)
    wk_d = nc.declare_dram_parameter("Wk", [D, D], F32R, isOutput=False)
    wv_d = nc.declare_dram_parameter("Wv", [D, D], F32R, isOutput=False)
    bv_d = nc.declare_dram_parameter("bv", [D], F32, isOutput=False)
    wo_d = nc.declare_dram_parameter("Wo", [D, D], F32R, isOutput=False)
    bo_d = nc.declare_dram_parameter("bo", [D], F32, isOutput=False)
    wres_d = nc.declare_dram_parameter("Wres", [DLAT, D], F32R, isOutput=False)
    bres_d = nc.declare_dram_parameter("bres", [D], F32, isOutput=False)
    lng_d = nc.declare_dram_parameter("ln_g", [D], F32, isOutput=False)
    lnb_d = nc.declare_dram_parameter("ln_b", [D], F32, isOutput=False)
    out_d = nc.declare_dram_parameter("out", [L, D], F32, isOutput=True)
    if debug_taps:
        dbg = {
            "dbg_xt": nc.declare_dram_parameter("dbg_xt", [128, 512], F32, isOutput=True),
            "dbg_kt": nc.declare_dram_parameter("dbg_kt", [128, 512], F32, isOutput=True),
            "dbg_v": nc.declare_dram_parameter("dbg_v", [128, 1024], F32, isOutput=True),
            "dbg_ex": nc.declare_dram_parameter("dbg_ex", [128, 512], F32, isOutput=True),
            "dbg_sall": nc.declare_dram_parameter("dbg_sall", [128, 8], F32, isOutput=True),
            "dbg_oacc": nc.declare_dram_parameter("dbg_oacc", [128, 1024], F32, isOutput=True),
            "dbg_y": nc.declare_dram_parameter("dbg_y", [65, 1024], F32, isOutput=True),
        }

    with tile.TileContext(nc) as tc, ExitStack() as ctx:
        const = ctx.enter_context(tc.tile_pool(name="const", bufs=1))
        xp = ctx.enter_context(tc.tile_pool(name="xp", bufs=4))
        xtp = ctx.enter_context(tc.tile_pool(name="xtp", bufs=1))
        vp = ctx.enter_context(tc.tile_pool(name="vp", bufs=1))
        ktp = ctx.enter_context(tc.tile_pool(name="ktp", bufs=2))
        ep = ctx.enter_context(tc.tile_pool(name="ep", bufs=2))
        atp = ctx.enter_context(tc.tile_pool(name="atp", bufs=2))
        rp = ctx.enter_context(tc.tile_pool(name="rp", bufs=1))
        pmm = ctx.enter_context(tc.tile_pool(name="pmm", bufs=4, space="PSUM"))
        ptb = ctx.enter_context(tc.tile_pool(name="ptb", bufs=2, space="PSUM"))
        py = ctx.enter_context(tc.tile_pool(name="py", bufs=1, space="PSUM"))

        # ---- constants ----
        ident = const.tile([128, 128], F32)
        masks.make_identity(nc, ident[:])
        identb = const.tile([128, 128], BF16)
        masks.make_identity(nc, identb[:])
        zero_b = const.tile([128, 1], F32)
        nc.vector.memset(zero_b[:], 0.0)
        eps_b = const.tile([L, 1], F32)
        nc.vector.memset(eps_b[:], LN_EPS)

        lat_sb = const.tile([L, DLAT], F32)
        nc.sync.dma_start(lat_sb[:], lat_d[:, :])
        wk_sb = const.tile([128, DB, D], F32R)
        nc.sync.dma_start(wk_sb[:], wk_d[:, :].rearrange("(i p) q -> p i q", p=128))
        wv_sb = const.tile([128, DB, D], F32R)
        nc.sync.dma_start(wv_sb[:], wv_d[:, :].rearrange("(i p) q -> p i q", p=128))
        wq_sb = const.tile([128, 4, D], F32R, tag="wqo")
        nc.sync.dma_start(wq_sb[:], wq_d[:, :].rearrange("(i p) q -> p i q", p=128))
        bq_sb = const.tile([128, 8], F32)
        nc.sync.dma_start(bq_sb[:], bq_d[:].rearrange("(a p) -> p a", p=128))
        bv_sb = const.tile([128, 8], F32)
        nc.sync.dma_start(bv_sb[:], bv_d[:].rearrange("(a p) -> p a", p=128))

        ones_t = const.tile([1, 128], F32)
        nc.vector.memset(ones_t[:], 1.0)

        # broadcast a [1, D] row to [L, D] via PE outer product ones.T @ row
        # (gpsimd partition_broadcast miscomputes for channels < 128)
        def bcast_sb_row(row_ap, full):
            for h in range(2):
                pb = pmm.tile([L, 512], F32, tag="mm")
                nc.tensor.matmul(
                    pb[:],
                    lhsT=ones_t[0:1, 0:L],
                    rhs=row_ap[0:1, h * 512:(h + 1) * 512],
                    start=True,
                    stop=True,
                )
                nc.vector.tensor_copy(full[:, h * 512:(h + 1) * 512], pb[:])

        # row vectors broadcast to 64 partitions (for free-dim adds/muls)
        def bcast_row(dram_ap, name):
            row = const.tile([1, D], F32)
            nc.sync.dma_start(row[:], dram_ap[:].rearrange("(a d) -> a d", a=1))
            full = const.tile([L, D], F32, tag=name)
            bcast_sb_row(row[:], full)
            return full

        bo_b = bcast_row(bo_d, "bo_b")
        bres_b = bcast_row(bres_d, "bres_b")
        lng_b = bcast_row(lng_d, "lng_b")
        lnb_b = bcast_row(lnb_d, "lnb_b")

        # ---- preamble: latT, qT, block-diagonal q ----
        latT = const.tile([128, 4, L], F32R)
        for c in range(4):
            ptp = pmm.tile([128, 128], F32, tag="mm")
            nc.tensor.transpose(
                ptp[:, 0:L], lat_sb[:, c * 128:(c + 1) * 128], ident[0:L, 0:L]
            )
            nc.vector.tensor_copy(latT[:, c, :], ptp[:, 0:L])

        bd_q = const.tile([128, NP, 128], F32R)
        zeros_t = const.tile([128, 128], F32)
        nc.vector.memset(zeros_t[:], 0.0)
        for p in range(NP):
            nc.vector.tensor_copy(bd_q[:, p, :], zeros_t[:])
        for p in range(NP):
            pq = pmm.tile([128, L], F32, tag="mm")
            for c in range(4):
                nc.tensor.matmul(
                    pq[:],
                    lhsT=(wq_sb[:, c, p * 128:(p + 1) * 128]),
                    rhs=(latT[:, c, :]),
                    start=(c == 0),
                    stop=(c == 3),
                )
            nc.vector.tensor_scalar_add(
                bd_q[0:64, p, 0:64], pq[0:64, :], bq_sb[0:64, p:p + 1]
            )
            nc.vector.tensor_scalar_add(
                bd_q[64:128, p, 64:128], pq[64:128, :], bq_sb[64:128, p:p + 1]
            )

        # ---- accumulators ----
        s_chunks = const.tile([128, NP, NCH], F32)
        out_acc = const.tile([128, NP, 128], F32)

        # ---- main loop over s-chunks ----
        for cc in range(NCH):
            xts = []
            for j in range(4):
                xt = xp.tile([128, D], F32, tag="x")
                nc.sync.dma_start(
                    xt[:], x_d[cc * SC + j * 128: cc * SC + (j + 1) * 128, :]
                )
                xts.append(xt)

            xT = xtp.tile([128, DB, SC], F32R, tag="xT")
            for j in range(4):
                for i in range(DB):
                    tp = pmm.tile([128, 128], F32, tag="mm")
                    nc.tensor.transpose(
                        tp[:], xts[j][:, i * 128:(i + 1) * 128], ident[:]
                    )
                    nc.vector.tensor_copy(
                        xT[:, i, j * 128:(j + 1) * 128], tp[:]
                    )

            if debug_taps and cc == 0:
                _t = const.tile([128, 1024], F32, tag="dbgscr", name="dbgscr")[:, 0:512]
                nc.vector.tensor_copy(_t[:], xT[:, 0, :].bitcast(F32))
                nc.sync.dma_start(dbg["dbg_xt"][:, :], _t[:])

            # V projection (bf16 out)
            vt = vp.tile([128, 4, D], BF16, tag="v")
            for j in range(4):
                for half in range(2):
                    pv = pmm.tile([128, 512], F32, tag="mm")
                    for i in range(DB):
                        nc.tensor.matmul(
                            pv[:],
                            lhsT=(xT[:, i, j * 128:(j + 1) * 128]),
                            rhs=(wv_sb[:, i, half * 512:(half + 1) * 512]),
                            start=(i == 0),
                            stop=(i == DB - 1),
                        )
                    nc.vector.tensor_copy(
                        vt[:, j, half * 512:(half + 1) * 512], pv[:]
                    )

            # per head-pair: K^T block, sim, exp, attn^T, AV
            for p in range(NP):
                pk = pmm.tile([128, 512], F32, tag="mm")
                for i in range(DB):
                    nc.tensor.matmul(
                        pk[:],
                        lhsT=(wk_sb[:, i, p * 128:(p + 1) * 128]),
                        rhs=(xT[:, i, :]),
                        start=(i == 0),
                        stop=(i == DB - 1),
                    )
                kt = ktp.tile([128, 512], F32R, tag="kt")
                nc.vector.tensor_copy(kt[:], pk[:])
                if debug_taps and cc == 0 and p == 0:
                    nc.sync.dma_start(dbg["dbg_kt"][:, :], kt[:].bitcast(F32))

                psim = pmm.tile([128, 512], F32, tag="mm")
                nc.tensor.matmul(
                    psim[:], lhsT=(bd_q[:, p, :]), rhs=(kt[:])
                )

                ex = ep.tile([128, 512], BF16, tag="exp")
                nc.scalar.activation(
                    ex[:], psim[:], AF.Exp, bias=zero_b[:], scale=SCALE,
                    accum_out=s_chunks[:, p, cc:cc + 1],
                )

                if debug_taps and cc == 0 and p == 0:
                    _e = const.tile([128, 1024], F32, tag="dbgscr", name="dbgscr")[:, 0:512]
                    nc.vector.tensor_copy(_e[:], ex[:])
                    nc.sync.dma_start(dbg["dbg_ex"][:, :], _e[:])
                att = atp.tile([128, 4, 128], BF16, tag="at")
                for j in range(4):
                    pt = ptb.tile([128, 128], BF16, tag="tb")
                    nc.tensor.transpose(
                        pt[:], ex[:, j * 128:(j + 1) * 128], identb[:]
                    )
                    nc.vector.tensor_copy(att[:, j, :], pt[:])

                pav = pmm.tile([128, 128], F32, tag="mm")
                for j in range(4):
                    nc.tensor.matmul(
                        pav[:],
                        lhsT=vt[:, j, p * 128:(p + 1) * 128],
                        rhs=att[:, j, :],
                        start=(j == 0),
                        stop=(j == 3),
                    )
                if cc == 0:
                    nc.vector.tensor_copy(out_acc[:, p, :], pav[:])
                else:
                    nc.vector.tensor_add(out_acc[:, p, :], out_acc[:, p, :], pav[:])
            if debug_taps and cc == 0:
                _v = const.tile([128, 1024], F32, tag="dbgscr", name="dbgscr")
                nc.vector.tensor_copy(_v[:], vt[:, 0, :])
                nc.sync.dma_start(dbg["dbg_v"][:, :], _v[:])

        # ---- epilogue ----
        # softmax denominators -> 1/S, transposed to free dim
        s_all = const.tile([128, NP], F32)
        nc.vector.tensor_reduce(s_all[:], s_chunks[:, :, :], axis=AX.X, op=OP.add)
        recip = const.tile([128, NP], F32)
        nc.vector.reciprocal(recip[:], s_all[:])
        if debug_taps:
            nc.sync.dma_start(dbg["dbg_sall"][:, :], s_all[:])
        recipST = const.tile([1, NP, 128], F32)
        for p in range(NP):
            ptr = pmm.tile([128, 128], F32, tag="mm")
            nc.tensor.transpose(ptr[0:1, :], recip[:, p:p + 1], ident[:])
            nc.vector.tensor_copy(recipST[0:1, p, :], ptr[0:1, :])

        olh = const.tile([128, NP, 65], F32R)
        for p in range(NP):
            R = rp.tile([128, 128], F32, tag="R")
            nc.gpsimd.partition_broadcast(R[:], recipST[0:1, p, :])
            nc.vector.tensor_mul(out_acc[:, p, :], out_acc[:, p, :], R[:])
            nc.vector.tensor_copy(olh[0:64, p, 0:64], out_acc[0:64, p, 0:64])
            nc.vector.tensor_copy(olh[64:128, p, 0:64], out_acc[64:128, p, 64:128])
            nc.vector.tensor_copy(olh[:, p, 64:65], bv_sb[:, p:p + 1])

        if debug_taps:
            _oa = const.tile([128, 1024], F32, tag="dbgscr", name="dbgscr")
            for p in range(NP):
                nc.vector.tensor_copy(_oa[:, p * 128:(p + 1) * 128], out_acc[:, p, :])
            nc.sync.dma_start(dbg["dbg_oacc"][:, :], _oa[:])

        # out-projection: y[65, 1024] (row 64 = bv @ Wo)
        wo_sb = const.tile([128, DB, D], F32R, tag="wqo")
        nc.sync.dma_start(wo_sb[:], wo_d[:, :].rearrange("(i p) q -> p i q", p=128))
        py_t = py.tile([65, D], F32, tag="y")
        for half in range(2):
            for p in range(NP):
                nc.tensor.matmul(
                    py_t[:, half * 512:(half + 1) * 512],
                    lhsT=(olh[:, p, :]),
                    rhs=(wo_sb[:, p, half * 512:(half + 1) * 512]),
                    start=(p == 0),
                    stop=(p == NP - 1),
                )
        y_sb = const.tile([65, D], F32)
        nc.vector.tensor_copy(y_sb[:], py_t[:])
        if debug_taps:
            nc.sync.dma_start(dbg["dbg_y"][:, :], y_sb[:])
        bvrow = const.tile([1, D], F32, tag="row")
        nc.sync.dma_start(bvrow[:], y_sb[64:65, :])
        bvwo = const.tile([L, D], F32)
        bcast_sb_row(bvrow[:], bvwo)
        y0 = y_sb[0:64, :]
        nc.vector.tensor_add(y0, y0, bvwo[:])
        nc.vector.tensor_add(y0, y0, bo_b[:])

        # layernorm
        mu = const.tile([L, 1], F32)
        nc.vector.tensor_reduce(mu[:], y0, axis=AX.X, op=OP.add)
        mus = const.tile([L, 1], F32)
        nc.scalar.mul(mus[:], mu[:], 1.0 / D)
        yc = const.tile([L, D], F32)
        nc.vector.tensor_scalar_sub(yc[:], y0, mus[:])
        sq = const.tile([L, D], F32)
        nc.vector.tensor_mul(sq[:], yc[:], yc[:])
        var = const.tile([L, 1], F32)
        nc.vector.tensor_reduce(var[:], sq[:], axis=AX.X, op=OP.add)
        std = const.tile([L, 1], F32)
        nc.scalar.activation(std[:], var[:], AF.Sqrt, bias=eps_b[:], scale=1.0 / D)
        rstd = const.tile([L, 1], F32)
        nc.vector.reciprocal(rstd[:], std[:])
        nc.vector.tensor_scalar_mul(yc[:], yc[:], rstd[:])
        nc.vector.tensor_mul(yc[:], yc[:], lng_b[:])
        nc.vector.tensor_add(yc[:], yc[:], lnb_b[:])

        # residual: latents @ Wres + bres
        wres_sb = const.tile([128, 4, D], F32R, tag="wqo")
        nc.sync.dma_start(
            wres_sb[:], wres_d[:, :].rearrange("(i p) q -> p i q", p=128)
        )
        pres = py.tile([L, D], F32, tag="y")
        for half in range(2):
            for c in range(4):
                nc.tensor.matmul(
                    pres[:, half * 512:(half + 1) * 512],
                    lhsT=(latT[:, c, :]),
                    rhs=(wres_sb[:, c, half * 512:(half + 1) * 512]),
                    start=(c == 0),
                    stop=(c == 3),
                )
        nc.vector.tensor_add(yc[:], yc[:], pres[:, :])
        nc.vector.tensor_add(yc[:], yc[:], bres_b[:])
        out_sb = const.tile([L, D], F32)
        nc.scalar.mul(out_sb[:], yc[:], RSQRT2)
        nc.sync.dma_start(out_d[:, :], out_sb[:])

    nc.compile()
    return nc


_NC_CACHE = None


def prepare_in_maps(inputs):
    x = np.ascontiguousarray(np.asarray(inputs["x"], dtype=np.float32))
    lat = np.ascontiguousarray(
        np.asarray(inputs["latents"], dtype=np.float32).reshape(L, DLAT)
    )
    common = {
        "latents": lat,
        "Wq": np.ascontiguousarray(np.asarray(inputs["Wq"], np.float32)),
        "bq": np.ascontiguousarray(np.asarray(inputs["bq"], np.float32)),
        "Wk": np.ascontiguousarray(np.asarray(inputs["Wk"], np.float32)),
        "Wv": np.ascontiguousarray(np.asarray(inputs["Wv"], np.float32)),
        "bv": np.ascontiguousarray(np.asarray(inputs["bv"], np.float32)),
        "Wo": np.ascontiguousarray(np.asarray(inputs["Wo"], np.float32)),
        "bo": np.ascontiguousarray(np.asarray(inputs["bo"], np.float32)),
        "Wres": np.ascontiguousarray(np.asarray(inputs["Wres"], np.float32)),
        "bres": np.ascontiguousarray(np.asarray(inputs["bres"], np.float32)),
        "ln_g": np.ascontiguousarray(np.asarray(inputs["ln_g"], np.float32)),
        "ln_b": np.ascontiguousarray(np.asarray(inputs["ln_b"], np.float32)),
    }
    return [dict(common, x=np.ascontiguousarray(x[b])) for b in range(N_CORES)]


def kernel(**inputs):
    global _NC_CACHE, LAST_RESULT
    if _NC_CACHE is None:
        _NC_CACHE = build_nc()
    nc = _NC_CACHE
    in_maps = prepare_in_maps(inputs)
    res = run_bass_kernel_spmd(nc, in_maps, list(range(N_CORES)))
    LAST_RESULT = res
    out = np.stack([np.asarray(res.results[b]["out"]) for b in range(N_CORES)])
    return out.astype(np.float32)
